# revision 7
# baseline (speedup 1.0000x reference)
"""Causal self-attention on 8 TRN2 NeuronCores.

Sharding: 4-way data parallel over batch x 2-way tensor parallel over heads.
Core c handles batch b=c//2, head group g=c%2 (heads 8g..8g+8).

Per-core device kernel (all matmuls bf16, fp32 PSUM accumulation):
  1. QKV projection from host-pretransposed xT [C, T]:
     - qT/kT produced head-dim-on-partitions ([128, T] tiles, head pairs)
     - V produced natural [T, 64/head] with an appended ones column (V')
  2. Causal attention per head, k-block-major:
     S^T[k,q] = K^T.T @ Q^T; diag mask add; exp on ACT (scale=1/8 folded);
     Y'[65, q] += V'_j.T @ expS^T accumulates unnormalized y^T AND the
     softmax denominator l (row 64, from the ones column).
     y^T = Y'[0:64] * (1/l) via DVE recip + rank-1 broadcast matmul.
  3. y^T lands in persistent SBUF tiles (SBUF->SBUF DMA); projection
     partial[q, :] = yT.T @ w_proj[group rows] + b_proj/2 over ALL q.
  4. Pairwise ReduceScatter(add) on bf16 partials sums the two head
     groups and hands each core its query half (rank index = parity, so
     the program stays SPMD-symmetric). Host concatenates 8 halves.
"""
import numpy as np
import ml_dtypes

B, T, C = 4, 2048, 1024
H = 16
D = C // H  # 64
HPC = 8            # heads per core
GD = HPC * D       # 512 dims per core's head group
NEG = -1.0e30

_CACHE = {}


def _build_nc():
    import concourse.bass as bass
    import concourse.mybir as mybir
    import concourse.tile as tile
    from concourse import bacc
    from contextlib import ExitStack

    f32 = mybir.dt.float32
    bf16 = mybir.dt.bfloat16

    nc = bacc.Bacc("TRN2", target_bir_lowering=False, debug=False, num_devices=8)

    xT = nc.declare_dram_parameter("xT", [C, T], bf16, isOutput=False)
    wq = nc.declare_dram_parameter("wq", [C, GD], bf16, isOutput=False)
    wk = nc.declare_dram_parameter("wk", [C, GD], bf16, isOutput=False)
    wv = nc.declare_dram_parameter("wv", [C, GD], bf16, isOutput=False)
    wp = nc.declare_dram_parameter("wp", [GD, C], bf16, isOutput=False)
    bq = nc.declare_dram_parameter("bq", [GD], f32, isOutput=False)
    bk = nc.declare_dram_parameter("bk", [GD], f32, isOutput=False)
    bv = nc.declare_dram_parameter("bv", [GD], f32, isOutput=False)
    bp = nc.declare_dram_parameter("bp", [C], f32, isOutput=False)
    out = nc.declare_dram_parameter("out", [T // 2, C], f32, isOutput=True)

    # ReduceScatter buffers: partial proj over all q -> own q half
    rs_in = nc.dram_tensor("rs_in", [T, C], bf16)
    rs_out = nc.dram_tensor("rs_out", [T // 2, C], bf16)

    NKB = T // 128   # 16 k-blocks per head
    NQC = T // 512   # 4 q-chunks of 512
    NCC = C // 128   # 8 contraction chunks

    with tile.TileContext(nc) as tc, ExitStack() as S0:
        consts = S0.enter_context(tc.tile_pool(name="consts", bufs=1))
        wp_pool = S0.enter_context(tc.tile_pool(name="wp", bufs=1))
        qk_pool = S0.enter_context(tc.tile_pool(name="qk", bufs=1))
        v_pool = S0.enter_context(tc.tile_pool(name="v", bufs=1))
        yt_pool = S0.enter_context(tc.tile_pool(name="yt", bufs=4))

        # ---- constants ----
        mask_ut = consts.tile([128, 128], f32, tag="mask")
        nc.gpsimd.memset(mask_ut, 0.0)
        # S^T[k, q] valid when k <= q: fill strict lower triangle (k > q)
        # with NEG.  predicate: -1*part + 1*free >= 0 keeps 0.0.
        nc.gpsimd.affine_select(
            out=mask_ut, in_=mask_ut,
            compare_op=mybir.AluOpType.is_ge, fill=NEG,
            base=0, pattern=[[1, 128]], channel_multiplier=-1,
        )
        ones_t = consts.tile([128, D], bf16, tag="ones")
        nc.vector.memset(ones_t, 1.0)
        bq_t = consts.tile([128, 4], f32, tag="bq")
        bk_t = consts.tile([128, 4], f32, tag="bk")
        for p in range(4):
            nc.sync.dma_start(
                out=bq_t[:, p : p + 1],
                in_=bq.ap()[128 * p : 128 * p + 128].rearrange("(p o) -> p o", o=1),
            )
            nc.sync.dma_start(
                out=bk_t[:, p : p + 1],
                in_=bk.ap()[128 * p : 128 * p + 128].rearrange("(p o) -> p o", o=1),
            )
        bv_bc = consts.tile([128, GD], f32, tag="bvb")
        nc.sync.dma_start(out=bv_bc, in_=bv.ap().partition_broadcast(128))
        bp_bc = consts.tile([128, C], f32, tag="bpb")
        nc.sync.dma_start(out=bp_bc, in_=bp.ap().partition_broadcast(128))

        # ---- persistent weights / activations ----
        wp_t = [wp_pool.tile([128, C], bf16, tag=f"wp{i}", name=f"wp{i}") for i in range(4)]
        for i in range(4):
            nc.sync.dma_start(out=wp_t[i], in_=wp.ap()[128 * i : 128 * i + 128, :])
        # persistent y^T tiles (filled during attention via SBUF->SBUF DMA)
        yf = [wp_pool.tile([128, T], bf16, tag=f"yf{p}", name=f"yf{p}") for p in range(4)]

        # qT/kT head-pair tiles: pair p partitions 0:64 = head 2p, 64:128 = 2p+1
        qT = [qk_pool.tile([128, T], bf16, tag=f"qT{p}", name=f"qT{p}") for p in range(4)]
        kT = [qk_pool.tile([128, T], bf16, tag=f"kT{p}", name=f"kT{p}") for p in range(4)]
        # V' tiles: per t-block [128, 8 heads x 65] (64 v-dims + ones col)
        vp = [v_pool.tile([128, HPC * 65], bf16, tag=f"vp{tb}", name=f"vp{tb}") for tb in range(NKB)]

        # ================= Phase B: QKV projection =================
        with ExitStack() as SB:
            xp = SB.enter_context(tc.tile_pool(name="xp", bufs=1))
            wqkv = SB.enter_context(tc.tile_pool(name="wqkv", bufs=1))
            psb = SB.enter_context(tc.tile_pool(name="psb", bufs=4, space="PSUM"))

            xT_t = [xp.tile([128, T], bf16, tag=f"x{i}", name=f"x{i}") for i in range(NCC)]
            for i in range(NCC):
                nc.sync.dma_start(out=xT_t[i], in_=xT.ap()[128 * i : 128 * i + 128, :])
            wq_t = [wqkv.tile([128, GD], bf16, tag=f"wq{i}", name=f"wqt{i}") for i in range(NCC)]
            wk_t = [wqkv.tile([128, GD], bf16, tag=f"wk{i}", name=f"wkt{i}") for i in range(NCC)]
            wv_t = [wqkv.tile([128, GD], bf16, tag=f"wv{i}", name=f"wvt{i}") for i in range(NCC)]
            for i in range(NCC):
                sl = slice(128 * i, 128 * i + 128)
                nc.sync.dma_start(out=wq_t[i], in_=wq.ap()[sl, :])
                nc.sync.dma_start(out=wk_t[i], in_=wk.ap()[sl, :])
                nc.sync.dma_start(out=wv_t[i], in_=wv.ap()[sl, :])

            # qT / kT: out [col-block 128, T] = w_chunk.T-free x xT
            for p in range(4):
                for which, w_t, b_col, dst in (
                    ("q", wq_t, bq_t[:, p : p + 1], qT[p]),
                    ("k", wk_t, bk_t[:, p : p + 1], kT[p]),
                ):
                    for t4 in range(4):  # T in 512 chunks
                        ps = psb.tile([128, 512], f32, tag="psqk")
                        for cc in range(NCC):
                            nc.tensor.matmul(
                                ps,
                                w_t[cc][:, 128 * p : 128 * p + 128],
                                xT_t[cc][:, 512 * t4 : 512 * t4 + 512],
                                start=(cc == 0), stop=(cc == NCC - 1),
                            )
                        nc.vector.tensor_scalar_add(
                            dst[:, 512 * t4 : 512 * t4 + 512], ps, b_col
                        )

            # V': out [t-block 128, 512 v-dims] -> strided into [128, 8, 65]
            for tb in range(NKB):
                ps = psb.tile([128, GD], f32, tag="psv")
                for cc in range(NCC):
                    nc.tensor.matmul(
                        ps,
                        xT_t[cc][:, 128 * tb : 128 * tb + 128],
                        wv_t[cc],
                        start=(cc == 0), stop=(cc == NCC - 1),
                    )
                v3 = vp[tb].rearrange("p (h e) -> p h e", e=65)
                nc.vector.tensor_add(
                    v3[:, :, 0:D],
                    ps.rearrange("p (h e) -> p h e", e=D),
                    bv_bc.rearrange("p (h e) -> p h e", e=D),
                )
                nc.vector.memset(v3[:, :, D : D + 1], 1.0)

        # ================= Phase C: attention =================
        with ExitStack() as SC:
            sps = SC.enter_context(tc.tile_pool(name="sps", bufs=2, space="PSUM"))
            yps = SC.enter_context(tc.tile_pool(name="yps", bufs=1, space="PSUM"))
            esp = SC.enter_context(tc.tile_pool(name="esp", bufs=3))
            rsp = SC.enter_context(tc.tile_pool(name="rsp", bufs=2))

            for h in range(HPC):
                p, r = h // 2, h % 2
                rb = slice(64 * r, 64 * r + 64)
                Y = [yps.tile([65, 512], f32, tag=f"y{c}", name=f"y{c}") for c in range(NQC)]
                for j in range(NKB):
                    ksl = slice(128 * j, 128 * j + 128)
                    for hf in range(j // 8, 2):
                        qa = max(128 * j, 1024 * hf)
                        qb = 1024 * (hf + 1)
                        st = sps.tile([128, 1024], f32, tag="s")
                        es = esp.tile([128, 1024], bf16, tag="es")
                        base = 1024 * hf
                        # S^T pieces, 512-bank-aligned
                        a = qa
                        while a < qb:
                            b_ = min(qb, 512 * (a // 512 + 1))
                            nc.tensor.matmul(
                                st[:, a - base : b_ - base],
                                kT[p][rb, ksl],
                                qT[p][rb, a:b_],
                                start=True, stop=True,
                            )
                            a = b_
                        if qa == 128 * j:  # diagonal block in this half
                            nc.vector.tensor_add(
                                st[:, qa - base : qa - base + 128],
                                st[:, qa - base : qa - base + 128],
                                mask_ut,
                            )
                        nc.scalar.activation(
                            es[:, qa - base : qb - base],
                            st[:, qa - base : qb - base],
                            mybir.ActivationFunctionType.Exp,
                            bias=0.0, scale=0.125,
                        )
                        # PV accumulation into Y chunks
                        a = qa
                        while a < qb:
                            b_ = min(qb, 512 * (a // 512 + 1))
                            c = a // 512
                            nc.tensor.matmul(
                                Y[c][:, a - 512 * c : b_ - 512 * c],
                                vp[j][:, 65 * h : 65 * h + 65],
                                es[:, a - base : b_ - base],
                                start=(j == 0),
                                stop=(j == min(NKB - 1, 4 * c + 3)),
                                skip_group_check=True,
                            )
                            a = b_
                # normalize and emit y^T
                for c in range(NQC):
                    rt = rsp.tile([65, 512], f32, tag="rt")
                    rbf = rsp.tile([65, 512], bf16, tag="rbf")
                    nc.vector.reciprocal(rt[64:65, :], Y[c][64:65, :])
                    nc.vector.tensor_copy(rbf[64:65, :], rt[64:65, :])
                    rbc = sps.tile([64, 512], f32, tag="s")
                    nc.tensor.matmul(
                        rbc, ones_t[64:65, 0:64], rbf[64:65, :],
                        start=True, stop=True,
                    )
                    rbs = rsp.tile([64, 512], f32, tag="rbs")
                    nc.vector.tensor_copy(rbs, rbc)
                    yts = yt_pool.tile([64, 512], bf16, tag="yts")
                    nc.vector.tensor_mul(yts, Y[c][0:64, :], rbs)
                    nc.sync.dma_start(
                        out=yf[p][rb, 512 * c : 512 * c + 512], in_=yts
                    )

        # ================= Phase D: projection + ReduceScatter =================
        with ExitStack() as SD:
            ob_pool = SD.enter_context(tc.tile_pool(name="ob", bufs=3))
            psd = SD.enter_context(tc.tile_pool(name="psd", bufs=4, space="PSUM"))

            for qq in range(T // 128):  # all 16 q-blocks
                ob = ob_pool.tile([128, C], bf16, tag="ob")
                for cc2 in range(2):
                    ps = psd.tile([128, 512], f32, tag="psd")
                    for dd in range(4):
                        nc.tensor.matmul(
                            ps,
                            yf[dd][:, 128 * qq : 128 * qq + 128],
                            wp_t[dd][:, 512 * cc2 : 512 * cc2 + 512],
                            start=(dd == 0), stop=(dd == 3),
                        )
                    nc.vector.tensor_add(
                        ob[:, 512 * cc2 : 512 * cc2 + 512],
                        ps,
                        bp_bc[:, 512 * cc2 : 512 * cc2 + 512],
                    )
                nc.sync.dma_start(
                    out=rs_in.ap()[128 * qq : 128 * qq + 128, :], in_=ob
                )

            nc.gpsimd.collective_compute(
                "ReduceScatter",
                mybir.AluOpType.add,
                ins=[rs_in.ap()],
                outs=[rs_out.ap()],
                replica_groups=[[0, 1], [2, 3], [4, 5], [6, 7]],
            )

            for qq in range(T // 256):  # 8 blocks of the owned half
                t_bf = ob_pool.tile([128, C], bf16, tag="tbf")
                t_f32 = ob_pool.tile([128, C], f32, tag="tf32")
                nc.sync.dma_start(
                    out=t_bf, in_=rs_out.ap()[128 * qq : 128 * qq + 128, :]
                )
                nc.vector.tensor_copy(t_f32, t_bf)
                nc.sync.dma_start(
                    out=out.ap()[128 * qq : 128 * qq + 128, :], in_=t_f32
                )

    nc.finalize()
    return nc


def get_nc():
    if "nc" not in _CACHE:
        _CACHE["nc"] = _build_nc()
    return _CACHE["nc"]


def build_in_maps(x, w_attn, b_attn, w_proj, b_proj):
    bf = ml_dtypes.bfloat16
    x = np.asarray(x, dtype=np.float32)
    w_attn = np.asarray(w_attn, dtype=np.float32)
    b_attn = np.asarray(b_attn, dtype=np.float32)
    w_proj = np.asarray(w_proj, dtype=np.float32)
    b_proj = np.asarray(b_proj, dtype=np.float32)

    in_maps = []
    for c in range(8):
        b, g = c // 2, c % 2
        sl = slice(GD * g, GD * g + GD)
        in_maps.append({
            "xT": np.ascontiguousarray(x[b].T).astype(bf),
            "wq": np.ascontiguousarray(w_attn[:, 0 * C :][:, sl]).astype(bf),
            "wk": np.ascontiguousarray(w_attn[:, 1 * C :][:, sl]).astype(bf),
            "wv": np.ascontiguousarray(w_attn[:, 2 * C :][:, sl]).astype(bf),
            "wp": np.ascontiguousarray(w_proj[GD * g : GD * g + GD, :]).astype(bf),
            "bq": np.ascontiguousarray(b_attn[0 * C :][sl]),
            "bk": np.ascontiguousarray(b_attn[1 * C :][sl]),
            "bv": np.ascontiguousarray(b_attn[2 * C :][sl]),
            "bp": (b_proj * 0.5).astype(np.float32),
        })

    return in_maps


def assemble_out(results):
    out = np.empty((B, T, C), dtype=np.float32)
    for c in range(8):
        b, m = c // 2, c % 2
        out[b, 1024 * m : 1024 * m + 1024, :] = results[c]["out"]
    return out


def kernel(x, w_attn, b_attn, w_proj, b_proj):
    from concourse.bass_utils import run_bass_kernel_spmd

    nc = get_nc()
    in_maps = build_in_maps(x, w_attn, b_attn, w_proj, b_proj)
    res = run_bass_kernel_spmd(nc, in_maps, core_ids=list(range(8)))
    return assemble_out(res.results)
